# revision 13
# baseline (speedup 1.0000x reference)
"""V3: time-split CRF forward kernel for Trainium2, constant-renorm edition.

Time-split sharding: each of 8 cores runs ALL 1024 batch elements over 1/8 of
the time axis (64 owned steps) plus a W=12-step warmup from uniform init (the
CRF recursion contracts to ~1e-8 in 12 steps) and one handoff step.  The host
stitches per-core scales with a telescoping recursion over the per-step z-rows.

Numeric range: instead of data-dependent renormalization, a constant per-step
log-shift c = 7*ln2 is baked into the features on the host (the per-step log
growth of this recursion is log-sum-exp dominated and empirically lives in
[4.1, 6.4], so a constant shift keeps |p| within e^±10 across any window).
Device corrections are the deterministic c*(i+1) — no renorm ops on device.

Within a core: 1024 elements packed 2-per-column (block-diagonal exp(trans))
into 512 columns, split into G=2 groups of 256 whose serial chains interleave
on PE/DVE.  Per step and group: one matmul [K=100 -> M=104, N=256] (rows
100..103 also produce d=EST.p and z=1.p) and one in-place DVE multiply
[104 x 256] into the marching ef buffer.  Ring-buffered ef tiles; d/z rows are
DMA'd out per tile.  All HBM layouts are group-major, every DMA contiguous.
"""

import sys

sys.path.insert(0, "/opt/trn_rl_repo")

import numpy as np

B, T, C = 1024, 512, 50
NCORES = 8
W = 12
NSTEP = W + 65               # 77 device steps per core
NCOLS = B // 2               # 512 columns
G = 2
GC = NCOLS // G              # 256 columns per group
ROWS = 104
RTILE = 7                    # steps per ring tile (77 = 7 * 11)
NTILE = NSTEP // RTILE
RB = 5                       # ring depth per group
GBLK = NSTEP * GC
CSHIFT = float(7 * np.log(2.0))

_cached = None


def _build_program():
    import concourse.bacc as bacc
    import concourse.tile as tile
    from concourse import mybir

    f32 = mybir.dt.float32
    nc = bacc.Bacc("TRN2", target_bir_lowering=False, debug=False)

    feats = nc.dram_tensor("feats", [ROWS, G * GBLK], f32, kind="ExternalInput")
    transT = nc.dram_tensor("transT", [C, C], f32, kind="ExternalInput")
    p0_in = nc.dram_tensor("p0_in", [100, NCOLS], f32, kind="ExternalInput")
    dzout = nc.dram_tensor("dzout", [4, G * GBLK], f32, kind="ExternalOutput")

    EXP = mybir.ActivationFunctionType.Exp

    with tile.TileContext(nc) as tc:
        with (
            tc.tile_pool(name="singles", bufs=1) as singles,
            tc.tile_pool(name="ring0", bufs=RB) as ring0,
            tc.tile_pool(name="ring1", bufs=RB) as ring1,
            tc.tile_pool(name="pmain0", bufs=4, space="PSUM") as pmain0,
            tc.tile_pool(name="pmain1", bufs=4, space="PSUM") as pmain1,
        ):
            rings = [ring0, ring1]
            pmains = [pmain0, pmain1]
            # --- constants (compute ops need 32-aligned partition starts;
            #     DMA is exempt, so build lhsT via an aligned staging tile) ---
            stg = singles.tile([64, 51], f32)
            nc.sync.dma_start(out=stg[0:50, 0:50], in_=transT[:, :])
            nc.scalar.activation(out=stg[0:50, 0:50], in_=stg[0:50, 0:50], func=EXP)
            nc.vector.memset(stg[0:50, 50:51], 1.0)

            lhsT = singles.tile([100, ROWS], f32)
            nc.vector.memset(lhsT, 0.0)
            nc.sync.dma_start(out=lhsT[0:50, 0:50], in_=stg[0:50, 0:50])
            nc.sync.dma_start(out=lhsT[50:100, 50:100], in_=stg[0:50, 0:50])
            nc.sync.dma_start(out=lhsT[0:50, 100:101], in_=stg[0:50, 49:50])
            nc.sync.dma_start(out=lhsT[50:100, 101:102], in_=stg[0:50, 49:50])
            nc.sync.dma_start(out=lhsT[0:50, 102:103], in_=stg[0:50, 50:51])
            nc.sync.dma_start(out=lhsT[50:100, 103:104], in_=stg[0:50, 50:51])

            p0 = singles.tile([100, NCOLS], f32)
            nc.sync.dma_start(out=p0[:, :], in_=p0_in[:, :])

            CHUNK = RTILE * GC
            tiles = [[] for _ in range(G)]

            def load_tile(g, k):
                t_ = rings[g].tile(
                    [ROWS, CHUNK], f32, name=f"ring{g}_t", tag=f"ring{g}_t"
                )
                base = g * GBLK + k * CHUNK
                nc.sync.dma_start(out=t_[:, :], in_=feats[:, base : base + CHUNK])
                nc.scalar.activation(out=t_[:, :], in_=t_[:, :], func=EXP)
                tiles[g].append(t_)

            for g in range(G):
                for k in range(min(2, NTILE)):
                    load_tile(g, k)

            for i in range(NSTEP):
                k, s = divmod(i, RTILE)
                for g in range(G):
                    if s == 0 and k + 2 < NTILE and k + 2 >= len(tiles[g]):
                        load_tile(g, k + 2)
                    cur = tiles[g][k]
                    if i == 0:
                        rhs = p0[:, g * GC : (g + 1) * GC]
                    else:
                        pk, psl = divmod(i - 1, RTILE)
                        rhs = tiles[g][pk][0:100, psl * GC : psl * GC + GC]
                    ps = pmains[g].tile([ROWS, GC], f32, name=f"ps{g}", tag=f"ps{g}")
                    nc.tensor.matmul(ps[:, :], lhsT[:, :], rhs, start=True, stop=True)
                    # p_{i+1} rows 0:100, d_i rows 100:102, z_i rows 102:104
                    efsl = cur[:, s * GC : (s + 1) * GC]
                    nc.vector.tensor_mul(efsl, ps[:, :], efsl)
                if s == RTILE - 1:
                    for g in range(G):
                        base = g * GBLK + k * CHUNK
                        nc.sync.dma_start(
                            out=dzout[:, base : base + CHUNK],
                            in_=tiles[g][k][100:104, :],
                        )

    nc.compile()
    return nc


def _get_program():
    global _cached
    if _cached is None:
        _cached = _build_program()
    return _cached


def _pack_feats_core(feats_full, c):
    """[B, T, C] f32 -> packed [104, G*NSTEP*GC] (group-major) for core c.

    The per-step constant log-shift is baked in: state rows get feat - c,
    d/z passthrough rows get -c (so every row of the multiply carries the
    same deterministic scale e^{-c} per step).
    """
    start = 0 if c == 0 else 64 * c - W
    ts = start + np.arange(NSTEP)
    valid = ts < T
    f = feats_full[:, np.minimum(ts, T - 1), :]
    f = f * valid[None, :, None] - CSHIFT                 # pad steps -> -c
    x = (
        f.astype(np.float32)
        .reshape(2, G, GC, NSTEP, C)
        .transpose(1, 0, 4, 3, 2)                         # [G, 2, C, NSTEP, GC]
        .reshape(G, 2 * C, NSTEP * GC)
    )
    out = np.full((ROWS, G * GBLK), np.float32(-CSHIFT), np.float32)
    for g in range(G):
        out[: 2 * C, g * GBLK : (g + 1) * GBLK] = x[g]
    return np.ascontiguousarray(out)


def kernel(lstm_feats, lens, transitions):
    from concourse.bass_utils import run_bass_kernel_spmd

    feats = np.ascontiguousarray(np.asarray(lstm_feats, dtype=np.float32))
    lens_np = np.asarray(lens).astype(np.int64)
    trans = np.asarray(transitions, dtype=np.float32)
    transT = np.ascontiguousarray(trans.T)

    p0_onehot = np.zeros((100, NCOLS), np.float32)
    p0_onehot[48, :] = 1.0
    p0_onehot[98, :] = 1.0
    p0_uniform = np.full((100, NCOLS), 1.0 / C, np.float32)

    nc = _get_program()
    in_maps = [
        {
            "feats": _pack_feats_core(feats, c),
            "transT": transT,
            "p0_in": p0_onehot if c == 0 else p0_uniform,
        }
        for c in range(NCORES)
    ]
    res = run_bass_kernel_spmd(nc, in_maps, list(range(NCORES)))
    global _last_exec_ns
    _last_exec_ns = res.exec_time_ns

    # ---- host assembly (O(B) bookkeeping) ----------------------------------
    bidx = np.arange(B)
    half = bidx // NCOLS
    grp = (bidx % NCOLS) // GC
    xcol = bidx % GC
    ii = np.arange(NSTEP)
    cols = grp[None, :] * GBLK + ii[:, None] * GC + xcol[None, :]

    dmat = np.zeros((NCORES, NSTEP, B), np.float64)
    zmat = np.zeros((NCORES, NSTEP, B), np.float64)
    for c in range(NCORES):
        dz = res.results[c]["dzout"]
        dmat[c] = dz[half[None, :], cols]
        zmat[c] = dz[2 + half[None, :], cols]

    logsig = np.zeros((NCORES, B))
    for c in range(1, NCORES):
        i_prev = 64 if c == 1 else 64 + W
        lam_prev = np.log(zmat[c - 1, i_prev]) + CSHIFT * (i_prev + 1)
        lam_cur = np.log(zmat[c, W]) + CSHIFT * (W + 1)
        logsig[c] = logsig[c - 1] + lam_prev - lam_cur

    owner = np.minimum(lens_np // 64, NCORES - 1).astype(np.int64)
    dev_i = np.where(owner == 0, lens_np, lens_np - (64 * owner - W))
    out = np.zeros(B, np.float64)
    for c in range(NCORES):
        m = owner == c
        if m.any():
            iim = dev_i[m]
            out[m] = (
                np.log(dmat[c, iim, m]) + CSHIFT * (iim + 1) + logsig[c, m]
            )
    return out.astype(np.float32)


# revision 15
# speedup vs baseline: 1.0661x; 1.0661x over previous
"""V3: time-split CRF forward kernel for Trainium2, constant-renorm edition.

Time-split sharding: each of 8 cores runs ALL 1024 batch elements over 1/8 of
the time axis (64 owned steps) plus a W=10-step warmup from uniform init (the
CRF recursion direction contracts ~0.27x/step from any init) and one handoff step.  The host
stitches per-core scales with a telescoping recursion over the per-step z-rows.

Numeric range: instead of data-dependent renormalization, a constant per-step
log-shift c = 7*ln2 is baked into the features on the host (the per-step log
growth of this recursion is log-sum-exp dominated and empirically lives in
[4.1, 6.4], so a constant shift keeps |p| within e^±10 across any window).
Device corrections are the deterministic c*(i+1) — no renorm ops on device.

Within a core: 1024 elements packed 2-per-column (block-diagonal exp(trans))
into 512 columns, split into G=2 groups of 256 whose serial chains interleave
on PE/DVE.  Per step and group: one matmul [K=100 -> M=104, N=256] (rows
100..103 also produce d=EST.p and z=1.p) and one in-place DVE multiply
[104 x 256] into the marching ef buffer.  Ring-buffered ef tiles; d/z rows are
DMA'd out per tile.  All HBM layouts are group-major, every DMA contiguous.
"""

import sys

sys.path.insert(0, "/opt/trn_rl_repo")

import numpy as np

B, T, C = 1024, 512, 50
NCORES = 8
W = 10
NSTEP = W + 65               # 75 device steps per core
NCOLS = B // 2               # 512 columns
G = 2
GC = NCOLS // G              # 256 columns per group
ROWS = 104
RTILE = 5                    # steps per ring tile (75 = 5 * 15)
NTILE = NSTEP // RTILE
RB = 5                       # ring depth per group
GBLK = NSTEP * GC
CSHIFT = float(7 * np.log(2.0))

_cached = None


def _build_program():
    import concourse.bacc as bacc
    import concourse.tile as tile
    from concourse import mybir

    f32 = mybir.dt.float32
    nc = bacc.Bacc("TRN2", target_bir_lowering=False, debug=False)

    feats = nc.dram_tensor("feats", [ROWS, G * GBLK], f32, kind="ExternalInput")
    transT = nc.dram_tensor("transT", [2 * C, C], f32, kind="ExternalInput")
    p0_in = nc.dram_tensor("p0_in", [100, NCOLS], f32, kind="ExternalInput")
    dzout = nc.dram_tensor("dzout", [4, G * GBLK], f32, kind="ExternalOutput")

    EXP = mybir.ActivationFunctionType.Exp

    with tile.TileContext(nc) as tc:
        with (
            tc.tile_pool(name="singles", bufs=1) as singles,
            tc.tile_pool(name="ring0", bufs=RB) as ring0,
            tc.tile_pool(name="ring1", bufs=RB) as ring1,
            tc.tile_pool(name="pmain0", bufs=4, space="PSUM") as pmain0,
            tc.tile_pool(name="pmain1", bufs=4, space="PSUM") as pmain1,
        ):
            rings = [ring0, ring1]
            pmains = [pmain0, pmain1]
            # --- constants (compute ops need 32-aligned partition starts;
            #     DMA is exempt).  transT arrives host-duplicated [100, 50] so
            #     one aligned exp covers both block-diagonal copies. ---
            stg = singles.tile([100, 51], f32)
            nc.sync.dma_start(out=stg[0:100, 0:50], in_=transT[:, :])
            nc.scalar.activation(
                out=stg[0:100, 0:50], in_=stg[0:100, 0:50], func=EXP
            )
            nc.vector.memset(stg[0:100, 50:51], 1.0)

            lhsT = singles.tile([100, ROWS], f32)
            nc.vector.memset(lhsT, 0.0)
            nc.sync.dma_start(out=lhsT[0:50, 0:50], in_=stg[0:50, 0:50])
            nc.sync.dma_start(out=lhsT[50:100, 50:100], in_=stg[50:100, 0:50])
            nc.sync.dma_start(out=lhsT[0:50, 100:101], in_=stg[0:50, 49:50])
            nc.sync.dma_start(out=lhsT[50:100, 101:102], in_=stg[50:100, 49:50])
            nc.sync.dma_start(out=lhsT[0:50, 102:103], in_=stg[0:50, 50:51])
            nc.sync.dma_start(out=lhsT[50:100, 103:104], in_=stg[50:100, 50:51])

            p0 = singles.tile([100, NCOLS], f32)
            nc.sync.dma_start(out=p0[:, :], in_=p0_in[:, :])

            CHUNK = RTILE * GC
            tiles = [[] for _ in range(G)]

            def load_tile(g, k):
                t_ = rings[g].tile(
                    [ROWS, CHUNK], f32, name=f"ring{g}_t", tag=f"ring{g}_t"
                )
                base = g * GBLK + k * CHUNK
                nc.sync.dma_start(out=t_[:, :], in_=feats[:, base : base + CHUNK])
                nc.scalar.activation(out=t_[:, :], in_=t_[:, :], func=EXP)
                tiles[g].append(t_)

            for g in range(G):
                for k in range(min(2, NTILE)):
                    load_tile(g, k)

            for i in range(NSTEP):
                k, s = divmod(i, RTILE)
                for g in range(G):
                    if s == 0 and k + 2 < NTILE and k + 2 >= len(tiles[g]):
                        load_tile(g, k + 2)
                    cur = tiles[g][k]
                    if i == 0:
                        rhs = p0[:, g * GC : (g + 1) * GC]
                    else:
                        pk, psl = divmod(i - 1, RTILE)
                        rhs = tiles[g][pk][0:100, psl * GC : psl * GC + GC]
                    ps = pmains[g].tile([ROWS, GC], f32, name=f"ps{g}", tag=f"ps{g}")
                    nc.tensor.matmul(ps[:, :], lhsT[:, :], rhs, start=True, stop=True)
                    # p_{i+1} rows 0:100, d_i rows 100:102, z_i rows 102:104
                    efsl = cur[:, s * GC : (s + 1) * GC]
                    nc.vector.tensor_mul(efsl, ps[:, :], efsl)
                if s == RTILE - 1:
                    for g in range(G):
                        base = g * GBLK + k * CHUNK
                        nc.sync.dma_start(
                            out=dzout[:, base : base + CHUNK],
                            in_=tiles[g][k][100:104, :],
                        )

    nc.compile()
    return nc


def _get_program():
    global _cached
    if _cached is None:
        _cached = _build_program()
    return _cached


def _pack_feats_core(feats_full, c):
    """[B, T, C] f32 -> packed [104, G*NSTEP*GC] (group-major) for core c.

    The per-step constant log-shift is baked in: state rows get feat - c,
    d/z passthrough rows get -c (so every row of the multiply carries the
    same deterministic scale e^{-c} per step).
    """
    start = 0 if c == 0 else 64 * c - W
    ts = start + np.arange(NSTEP)
    valid = ts < T
    f = feats_full[:, np.minimum(ts, T - 1), :]
    f = f * valid[None, :, None] - CSHIFT                 # pad steps -> -c
    x = (
        f.astype(np.float32)
        .reshape(2, G, GC, NSTEP, C)
        .transpose(1, 0, 4, 3, 2)                         # [G, 2, C, NSTEP, GC]
        .reshape(G, 2 * C, NSTEP * GC)
    )
    out = np.full((ROWS, G * GBLK), np.float32(-CSHIFT), np.float32)
    for g in range(G):
        out[: 2 * C, g * GBLK : (g + 1) * GBLK] = x[g]
    return np.ascontiguousarray(out)


def kernel(lstm_feats, lens, transitions):
    from concourse.bass_utils import run_bass_kernel_spmd

    feats = np.ascontiguousarray(np.asarray(lstm_feats, dtype=np.float32))
    lens_np = np.asarray(lens).astype(np.int64)
    trans = np.asarray(transitions, dtype=np.float32)
    transT = np.ascontiguousarray(np.vstack([trans.T, trans.T]))

    p0_onehot = np.zeros((100, NCOLS), np.float32)
    p0_onehot[48, :] = 1.0
    p0_onehot[98, :] = 1.0
    p0_uniform = np.full((100, NCOLS), 1.0 / C, np.float32)

    nc = _get_program()
    in_maps = [
        {
            "feats": _pack_feats_core(feats, c),
            "transT": transT,
            "p0_in": p0_onehot if c == 0 else p0_uniform,
        }
        for c in range(NCORES)
    ]
    res = run_bass_kernel_spmd(nc, in_maps, list(range(NCORES)))
    global _last_exec_ns
    _last_exec_ns = res.exec_time_ns

    # ---- host assembly (O(B) bookkeeping) ----------------------------------
    bidx = np.arange(B)
    half = bidx // NCOLS
    grp = (bidx % NCOLS) // GC
    xcol = bidx % GC
    ii = np.arange(NSTEP)
    cols = grp[None, :] * GBLK + ii[:, None] * GC + xcol[None, :]

    dmat = np.zeros((NCORES, NSTEP, B), np.float64)
    zmat = np.zeros((NCORES, NSTEP, B), np.float64)
    for c in range(NCORES):
        dz = res.results[c]["dzout"]
        dmat[c] = dz[half[None, :], cols]
        zmat[c] = dz[2 + half[None, :], cols]

    logsig = np.zeros((NCORES, B))
    for c in range(1, NCORES):
        i_prev = 64 if c == 1 else 64 + W
        lam_prev = np.log(zmat[c - 1, i_prev]) + CSHIFT * (i_prev + 1)
        lam_cur = np.log(zmat[c, W]) + CSHIFT * (W + 1)
        logsig[c] = logsig[c - 1] + lam_prev - lam_cur

    owner = np.minimum(lens_np // 64, NCORES - 1).astype(np.int64)
    dev_i = np.where(owner == 0, lens_np, lens_np - (64 * owner - W))
    out = np.zeros(B, np.float64)
    for c in range(NCORES):
        m = owner == c
        if m.any():
            iim = dev_i[m]
            out[m] = (
                np.log(dmat[c, iim, m]) + CSHIFT * (iim + 1) + logsig[c, m]
            )
    return out.astype(np.float32)


# revision 16
# speedup vs baseline: 1.0714x; 1.0050x over previous
"""V3: time-split CRF forward kernel for Trainium2, constant-renorm edition.

Time-split sharding: each of 8 cores runs ALL 1024 batch elements over 1/8 of
the time axis (64 owned steps) plus a W=10-step warmup from uniform init (the
CRF recursion direction contracts ~0.27x/step from any init) and one handoff step.  The host
stitches per-core scales with a telescoping recursion over the per-step z-rows.

Numeric range: instead of data-dependent renormalization, a constant per-step
log-shift c = 7*ln2 is baked into the features on the host (the per-step log
growth of this recursion is log-sum-exp dominated and empirically lives in
[4.1, 6.4], so a constant shift keeps |p| within e^±10 across any window).
Device corrections are the deterministic c*(i+1) — no renorm ops on device.

Within a core: 1024 elements packed 2-per-column (block-diagonal exp(trans))
into 512 columns, split into G=2 groups of 256 whose serial chains interleave
on PE/DVE.  Per step and group: one matmul [K=100 -> M=104, N=256] (rows
100..103 also produce d=EST.p and z=1.p) and one in-place DVE multiply
[104 x 256] into the marching ef buffer.  Ring-buffered ef tiles; d/z rows are
DMA'd out per tile.  All HBM layouts are group-major, every DMA contiguous.
"""

import sys

sys.path.insert(0, "/opt/trn_rl_repo")

import numpy as np

B, T, C = 1024, 512, 50
NCORES = 8
W = 10
NSTEP = W + 65               # 75 device steps per core
NCOLS = B // 2               # 512 columns
G = 2
GC = NCOLS // G              # 256 columns per group
ROWS = 104
RTILE = 5                    # steps per ring tile (75 = 5 * 15)
NTILE = NSTEP // RTILE
RB = 5                       # ring depth per group
GBLK = NSTEP * GC
CSHIFT = float(7 * np.log(2.0))

_cached = None


def _build_program():
    import concourse.bacc as bacc
    import concourse.tile as tile
    from concourse import mybir

    f32 = mybir.dt.float32
    nc = bacc.Bacc("TRN2", target_bir_lowering=False, debug=False)

    feats = nc.dram_tensor("feats", [ROWS, G * GBLK], f32, kind="ExternalInput")
    transT = nc.dram_tensor("transT", [2 * C, C], f32, kind="ExternalInput")
    p0_in = nc.dram_tensor("p0_in", [100, NCOLS], f32, kind="ExternalInput")
    dzout = nc.dram_tensor("dzout", [4, G * GBLK], f32, kind="ExternalOutput")

    EXP = mybir.ActivationFunctionType.Exp

    with tile.TileContext(nc) as tc:
        with (
            tc.tile_pool(name="singles", bufs=1) as singles,
            tc.tile_pool(name="ring0", bufs=RB) as ring0,
            tc.tile_pool(name="ring1", bufs=RB) as ring1,
            tc.tile_pool(name="pmain0", bufs=4, space="PSUM") as pmain0,
            tc.tile_pool(name="pmain1", bufs=4, space="PSUM") as pmain1,
        ):
            rings = [ring0, ring1]
            pmains = [pmain0, pmain1]
            # --- constants (compute ops need 32-aligned partition starts;
            #     DMA is exempt).  transT arrives host-duplicated [100, 50] so
            #     one aligned exp covers both block-diagonal copies. ---
            stg = singles.tile([100, 51], f32)
            nc.sync.dma_start(out=stg[0:100, 0:50], in_=transT[:, :])
            nc.scalar.activation(
                out=stg[0:100, 0:50], in_=stg[0:100, 0:50], func=EXP
            )
            nc.vector.memset(stg[0:100, 50:51], 1.0)

            lhsT = singles.tile([100, ROWS], f32)
            nc.vector.memset(lhsT, 0.0)
            nc.sync.dma_start(out=lhsT[0:50, 0:50], in_=stg[0:50, 0:50])
            nc.sync.dma_start(out=lhsT[50:100, 50:100], in_=stg[50:100, 0:50])
            nc.sync.dma_start(out=lhsT[0:50, 100:101], in_=stg[0:50, 49:50])
            nc.sync.dma_start(out=lhsT[50:100, 101:102], in_=stg[50:100, 49:50])
            nc.sync.dma_start(out=lhsT[0:50, 102:103], in_=stg[0:50, 50:51])
            nc.sync.dma_start(out=lhsT[50:100, 103:104], in_=stg[50:100, 50:51])

            p0 = singles.tile([100, NCOLS], f32)
            nc.sync.dma_start(out=p0[:, :], in_=p0_in[:, :])

            CHUNK = RTILE * GC
            tiles = [[] for _ in range(G)]

            def load_tile(g, k):
                t_ = rings[g].tile(
                    [ROWS, CHUNK], f32, name=f"ring{g}_t", tag=f"ring{g}_t"
                )
                base = g * GBLK + k * CHUNK
                nc.sync.dma_start(out=t_[:, :], in_=feats[:, base : base + CHUNK])
                nc.scalar.activation(out=t_[:, :], in_=t_[:, :], func=EXP)
                tiles[g].append(t_)

            for g in range(G):
                for k in range(min(3, NTILE)):
                    load_tile(g, k)

            for i in range(NSTEP):
                k, s = divmod(i, RTILE)
                for g in range(G):
                    if s == 0 and k + 3 < NTILE and k + 3 >= len(tiles[g]):
                        load_tile(g, k + 3)
                    cur = tiles[g][k]
                    if i == 0:
                        rhs = p0[:, g * GC : (g + 1) * GC]
                    else:
                        pk, psl = divmod(i - 1, RTILE)
                        rhs = tiles[g][pk][0:100, psl * GC : psl * GC + GC]
                    ps = pmains[g].tile([ROWS, GC], f32, name=f"ps{g}", tag=f"ps{g}")
                    nc.tensor.matmul(ps[:, :], lhsT[:, :], rhs, start=True, stop=True)
                    # p_{i+1} rows 0:100, d_i rows 100:102, z_i rows 102:104
                    efsl = cur[:, s * GC : (s + 1) * GC]
                    nc.vector.tensor_mul(efsl, ps[:, :], efsl)
                if s == RTILE - 1:
                    for g in range(G):
                        base = g * GBLK + k * CHUNK
                        nc.sync.dma_start(
                            out=dzout[:, base : base + CHUNK],
                            in_=tiles[g][k][100:104, :],
                        )

    nc.compile()
    return nc


def _get_program():
    global _cached
    if _cached is None:
        _cached = _build_program()
    return _cached


def _pack_feats_core(feats_full, c):
    """[B, T, C] f32 -> packed [104, G*NSTEP*GC] (group-major) for core c.

    The per-step constant log-shift is baked in: state rows get feat - c,
    d/z passthrough rows get -c (so every row of the multiply carries the
    same deterministic scale e^{-c} per step).
    """
    start = 0 if c == 0 else 64 * c - W
    ts = start + np.arange(NSTEP)
    valid = ts < T
    f = feats_full[:, np.minimum(ts, T - 1), :]
    f = f * valid[None, :, None] - CSHIFT                 # pad steps -> -c
    x = (
        f.astype(np.float32)
        .reshape(2, G, GC, NSTEP, C)
        .transpose(1, 0, 4, 3, 2)                         # [G, 2, C, NSTEP, GC]
        .reshape(G, 2 * C, NSTEP * GC)
    )
    out = np.full((ROWS, G * GBLK), np.float32(-CSHIFT), np.float32)
    for g in range(G):
        out[: 2 * C, g * GBLK : (g + 1) * GBLK] = x[g]
    return np.ascontiguousarray(out)


def kernel(lstm_feats, lens, transitions):
    from concourse.bass_utils import run_bass_kernel_spmd

    feats = np.ascontiguousarray(np.asarray(lstm_feats, dtype=np.float32))
    lens_np = np.asarray(lens).astype(np.int64)
    trans = np.asarray(transitions, dtype=np.float32)
    transT = np.ascontiguousarray(np.vstack([trans.T, trans.T]))

    p0_onehot = np.zeros((100, NCOLS), np.float32)
    p0_onehot[48, :] = 1.0
    p0_onehot[98, :] = 1.0
    p0_uniform = np.full((100, NCOLS), 1.0 / C, np.float32)

    nc = _get_program()
    in_maps = [
        {
            "feats": _pack_feats_core(feats, c),
            "transT": transT,
            "p0_in": p0_onehot if c == 0 else p0_uniform,
        }
        for c in range(NCORES)
    ]
    res = run_bass_kernel_spmd(nc, in_maps, list(range(NCORES)))
    global _last_exec_ns
    _last_exec_ns = res.exec_time_ns

    # ---- host assembly (O(B) bookkeeping) ----------------------------------
    bidx = np.arange(B)
    half = bidx // NCOLS
    grp = (bidx % NCOLS) // GC
    xcol = bidx % GC
    ii = np.arange(NSTEP)
    cols = grp[None, :] * GBLK + ii[:, None] * GC + xcol[None, :]

    dmat = np.zeros((NCORES, NSTEP, B), np.float64)
    zmat = np.zeros((NCORES, NSTEP, B), np.float64)
    for c in range(NCORES):
        dz = res.results[c]["dzout"]
        dmat[c] = dz[half[None, :], cols]
        zmat[c] = dz[2 + half[None, :], cols]

    logsig = np.zeros((NCORES, B))
    for c in range(1, NCORES):
        i_prev = 64 if c == 1 else 64 + W
        lam_prev = np.log(zmat[c - 1, i_prev]) + CSHIFT * (i_prev + 1)
        lam_cur = np.log(zmat[c, W]) + CSHIFT * (W + 1)
        logsig[c] = logsig[c - 1] + lam_prev - lam_cur

    owner = np.minimum(lens_np // 64, NCORES - 1).astype(np.int64)
    dev_i = np.where(owner == 0, lens_np, lens_np - (64 * owner - W))
    out = np.zeros(B, np.float64)
    for c in range(NCORES):
        m = owner == c
        if m.any():
            iim = dev_i[m]
            out[m] = (
                np.log(dmat[c, iim, m]) + CSHIFT * (iim + 1) + logsig[c, m]
            )
    return out.astype(np.float32)
